# revision 11
# baseline (speedup 1.0000x reference)
"""BigBird attention kernel for 8 Trainium2 NeuronCores.

Sharding: data-parallel over batch (2) x tensor-parallel over heads (4 groups
of 4 heads) = 8 cores. Each core computes q/k/v projections for its head
slice, block-sparse masked attention, and a partial output projection with
its Wo row-slice; the host sums the 4 partial outputs per batch.

v3 design (vs the ~148us phase-serial baseline):
- Fused pipeline: program order qkv(0), qkv(1), attn(q0-3), qkv(2),
  attn(q4-7), qkv(3), attn(q8-11), attn(q12-13), attn(q14), attn(q15).
  ACT-engine exp overlaps PE QKV matmuls (bigbird mask is block-causal);
  the tapered attention chunks keep the un-overlappable tail small.
- PE warmup: a dozen matmuls on a zero tile ramp the tensor engine's
  p-state and cover the input-DMA window before real work arrives.
- No identity-matmul mask preload. The mask is applied as a post-exp 0/1
  multiply on DVE; exp inputs are ~N(0,1) so no -inf bias is needed.
- Column-uniform masked blocks (the global k<16 column for q-tiles >= 3)
  skip the DVE mask: their AV matmuls use a copy of v[kt] with the
  masked k-rows (and the ones column) zeroed.
- AV runs at chunk granularity: one matmul per (kt, q-range) chunk with
  up to 512 moving columns (vs per-q-tile 128-col matmuls that ran far
  below PE peak). Every q-tile's accumulation starts at the global kt=0
  chunk, so start flags stay consistent across regions.
- Scores contract 64 real head-dim partitions (no zero-padded k tiles).
- Host-packed [128, chunk, cols] inputs: one DMA per tensor (descriptor
  gen costs ~0.6us engine time per dma_start); x per 512-col band, band
  0 in 4 pieces. Issue order puts first-band operands first.
- PSUM budget (8 banks of [128,512]f32): qk x2, rot x1, v x1, scores x2,
  av/Wo shared tag x2.
- Scores are computed TRANSPOSED (S^T[k,q]); P^T feeds AV as the moving
  operand with V plus a ones column (softmax row-sum l for free)
  stationary. 1/l via fast-approx DVE reciprocal, GpSimd
  partition_broadcast, folded into the psum->sbuf copy of O^T.
- Rope combine (m = raw*cos, out = m + u) runs on GpSimd; only the
  psum-reading u = rot*sin stays on DVE.
"""

import sys

for _p in ("/opt/trn_rl_repo", "/opt/trn_rl_repo/concourse"):
    if _p not in sys.path:
        sys.path.insert(0, _p)

import numpy as np

import concourse.bacc as bacc
import concourse.bass as bass
import concourse.mybir as mybir
import concourse.tile as tile
from concourse import bass_utils

F32 = mybir.dt.float32
BF16 = mybir.dt.bfloat16

B, S, D, H = 2, 2048, 1024, 16
HD = D // H          # 64
SCALE = 1.0 / float(np.sqrt(HD))
NCORES = 8
HG = 4               # head groups (tensor-parallel)
HPC = H // HG        # heads per core = 4
DC = HPC * HD        # channels per core = 256
QT = 128             # supertile edge
NQ = S // QT         # 16
NG = 4               # qkv bands of 512 seq positions
KC = D // 128        # 8 contraction chunks
CC = DC // 128       # 2 channel chunks (2 heads each)

# attention chunk boundaries (q-tile lo, hi-exclusive) and how many heads
# of scores to run ahead of AV in each chunk
AQ = [(0, 4, 2), (4, 8, 2), (8, 12, 2), (12, 14, 4), (14, 15, 4),
      (15, 16, 4)]


def _runs(bools):
    """Maximal [lo, hi) runs of True."""
    out = []
    lo = None
    for i, b in enumerate(list(bools) + [False]):
        if b and lo is None:
            lo = i
        elif not b and lo is not None:
            out.append((lo, i))
            lo = None
    return out


def _sched(mask):
    """Block-sparse schedule + mask/variant metadata from the runtime mask."""
    sup = mask.reshape(NQ, QT, NQ, QT).any(axis=(1, 3))  # [16,16]
    kts = [np.nonzero(sup[qi])[0].tolist() for qi in range(NQ)]
    kset = sorted({kt for qi in range(NQ) for kt in kts[qi]})
    ulo, uhi = {}, {}
    for kt in kset:
        us = [qi for qi in range(NQ) if kt in kts[qi]]
        ulo[kt], uhi[kt] = min(us), max(us)
    kts_eff = [[kt for kt in kset if ulo[kt] <= qi <= uhi[kt]]
               for qi in range(NQ)]
    assert all(kts_eff[qi] for qi in range(NQ)), "fully masked q row"
    # chunk-granular AV needs every q-tile's chain to start on the same
    # (first) k-tile; the global column guarantees it for bigbird
    k0 = kset[0]
    assert all(kts_eff[qi][0] == k0 and ulo[k0] == 0 and uhi[k0] == NQ - 1
               for qi in range(NQ)), "no common leading k-tile"

    # column-uniform blocks -> v-variant candidates
    pats = {}
    for qi in range(NQ):
        for kt in kts_eff[qi]:
            blk = mask[qi * QT:(qi + 1) * QT, kt * QT:(kt + 1) * QT]
            if blk.all():
                continue  # fully dense: nothing to mask anyway
            if np.all(blk == blk[0:1, :]):
                pat = blk[0]
                rr = _runs(pat)
                if 1 <= len(rr) <= 2:
                    pats.setdefault((kt, pat.tobytes()), (rr, set()))[1].add(qi)
    variants, var_of = [], {}
    for (kt, _pb), (rr, users) in sorted(pats.items(), key=lambda x: x[0][0]):
        if len(users) >= 2 and kt < 4:
            vi = len(variants)
            variants.append((kt, rr, users))
            for qi in users:
                var_of[(qi, kt)] = vi

    chunks = []
    mask_blocks = []  # (qi, kt) in packed order
    mcols = 0
    pb_need = 0
    for qlo0, qhi0, ahead in AQ:
        bchunks = []
        boff = 0
        for kt in kset:
            qlo = max(ulo[kt], qlo0)
            qhi = min(uhi[kt], qhi0 - 1)
            if qlo > qhi:
                continue
            W = (qhi - qlo + 1) * QT
            bchunks.append((kt, qlo, qhi, W, boff))
            boff += W
        assert bchunks[0][0] == k0 and bchunks[0][1] == qlo0 \
            and bchunks[0][2] == qhi0 - 1
        groups = []
        cur, curw = [], 0
        for ch in bchunks:
            if curw + ch[3] > 512:
                groups.append(cur)
                cur, curw = [], 0
            cur.append(ch)
            curw += ch[3]
        if cur:
            groups.append(cur)
        pb_need = max(pb_need, (min(ahead, HPC) + 1) * len(groups))

        # per-group mask runs + kt -> (group, offset, qlo) map for AV
        moff0 = mcols
        bw = 0
        gruns = []
        ktmap = {}
        for gi, grp in enumerate(groups):
            g0 = grp[0][4]
            runs = []
            cur_run = None
            for kt, qlo, qhi, W, bo in grp:
                ktmap[kt] = (gi, bo - g0, qlo)
                for qi in range(qlo, qhi + 1):
                    go = bo - g0 + (qi - qlo) * QT
                    blk = mask[qi * QT:(qi + 1) * QT, kt * QT:(kt + 1) * QT]
                    vi = var_of.get((qi, kt))
                    need = not blk.all() and vi is None
                    if need:
                        mask_blocks.append((qi, kt))
                        if cur_run is not None and \
                                cur_run[0] + cur_run[1] == go:
                            cur_run = (cur_run[0], cur_run[1] + QT,
                                       cur_run[2])
                        else:
                            if cur_run is not None:
                                runs.append(cur_run)
                            cur_run = (go, QT, bw)
                        bw += QT
                    else:
                        if cur_run is not None:
                            runs.append(cur_run)
                            cur_run = None
            if cur_run is not None:
                runs.append(cur_run)
            gruns.append(runs)
        mcols += bw
        chunks.append(dict(qlo=qlo0, qhi=qhi0, ahead=ahead, groups=groups,
                           mask_runs=gruns, mask_off=moff0, mask_w=bw,
                           ktmap=ktmap))

    return dict(kts_eff=kts_eff, chunks=chunks, mask_cols=mcols,
                mask_blocks=mask_blocks, variants=variants, var_of=var_of,
                pb_bufs=pb_need + 2)


def _build_nc(sc):
    chunks = sc["chunks"]
    kts_eff = sc["kts_eff"]
    nvar = len(sc["variants"])

    nc = bacc.Bacc("TRN2", target_bir_lowering=False, debug=False)

    xc_d = nc.dram_tensor("xc", [128, KC, S], BF16, kind="ExternalInput")
    wq_d = nc.dram_tensor("wq", [128, KC, DC], BF16, kind="ExternalInput")
    wk_d = nc.dram_tensor("wk", [128, KC, DC], BF16, kind="ExternalInput")
    wv_d = nc.dram_tensor("wv", [128, KC, DC], BF16, kind="ExternalInput")
    wo_d = nc.dram_tensor("wo", [128, CC, D], BF16, kind="ExternalInput")
    cos_d = nc.dram_tensor("cosT", [128, S], BF16, kind="ExternalInput")
    sin_d = nc.dram_tensor("sinT", [128, S], BF16, kind="ExternalInput")
    rt_d = nc.dram_tensor("rT", [128, 128], BF16, kind="ExternalInput")
    mcols = max(sc["mask_cols"], 128)
    mk_d = nc.dram_tensor("maskT", [128, mcols], BF16, kind="ExternalInput")
    out_d = nc.dram_tensor("out", [S, D], BF16, kind="ExternalOutput")

    with tile.TileContext(nc) as tc:
        from contextlib import ExitStack
        ctx = ExitStack()
        pp = ctx.enter_context(tc.tile_pool(name="persist", bufs=1))
        wp = ctx.enter_context(tc.tile_pool(name="weights", bufs=1))
        xp = ctx.enter_context(tc.tile_pool(name="xchunks", bufs=1))
        mp = ctx.enter_context(tc.tile_pool(name="maskp", bufs=1))
        sp = ctx.enter_context(tc.tile_pool(name="scratch", bufs=3))
        bp = ctx.enter_context(tc.tile_pool(name="probs",
                                            bufs=sc["pb_bufs"]))
        lr = ctx.enter_context(tc.tile_pool(name="lrec", bufs=2))
        otp = ctx.enter_context(tc.tile_pool(name="otile", bufs=2))
        obp = ctx.enter_context(tc.tile_pool(name="obuf", bufs=3))
        ps = ctx.enter_context(tc.tile_pool(name="psum", bufs=1,
                                            space="PSUM"))

        # ---- persistent tiles ----
        qbT = [pp.tile([128, S], BF16, tag=f"qbT{c}", name=f"qbT{c}")
               for c in range(CC)]
        kbT = [pp.tile([128, S], BF16, tag=f"kbT{c}", name=f"kbT{c}")
               for c in range(CC)]
        vb1 = [pp.tile([128, HPC, HD + 1], BF16, tag=f"v{i}", name=f"v{i}")
               for i in range(NQ)]
        vgs = [pp.tile([128, HPC, HD + 1], BF16, tag=f"vg{j}", name=f"vg{j}")
               for j in range(nvar)]

        wq_sb = wp.tile([128, KC, DC], BF16, tag="wq")
        wk_sb = wp.tile([128, KC, DC], BF16, tag="wk")
        wv_sb = wp.tile([128, KC, DC], BF16, tag="wv")
        wo_sb = wp.tile([128, CC, D], BF16, tag="wo")
        cosT = wp.tile([128, S], BF16, tag="cosT")
        sinT = wp.tile([128, S], BF16, tag="sinT")
        rT = wp.tile([128, 128], BF16, tag="rT")
        warm = wp.tile([128, 512], BF16, tag="warm")

        xb = [xp.tile([128, KC, 512], BF16, tag=f"xb{pc}",
                      name=f"xb{pc}") for pc in range(NG)]
        mbs = [mp.tile([128, max(ch["mask_w"], 128)], BF16, tag=f"mb{i}",
                       name=f"mb{i}") for i, ch in enumerate(chunks)]

        # ---- PE warmup: ramp the p-state, cover the DMA window ----
        nc.vector.memset(warm[:], 0.0)
        for w in range(12):
            pswarm = ps.tile([128, 512], F32, tag="qk", bufs=2,
                             name="pswarm")
            nc.tensor.matmul(pswarm[:], warm[:, 0:128], warm[:],
                             start=True, stop=True)

        # ---- upfront DMA issue, spread across the 3 dma-capable queues ----
        # scalar: wq/wk first (chain operands), wv, x bands 1+3
        nc.scalar.dma_start(wq_sb[:], wq_d[:, :, :])
        # sync: x band0 in 4 pieces (PE starts after the first)
        for j in range(4):
            nc.sync.dma_start(xb[0][:, 2 * j:2 * j + 2, :],
                              xc_d[:, 2 * j:2 * j + 2, 0:512])
        nc.scalar.dma_start(wk_sb[:], wk_d[:, :, :])
        nc.scalar.dma_start(wv_sb[:], wv_d[:, :, :])
        # gpsimd: rope tables
        nc.gpsimd.dma_start(rT[:], rt_d[:, :])
        nc.gpsimd.dma_start(cosT[:], cos_d[:, :])
        nc.gpsimd.dma_start(sinT[:], sin_d[:, :])
        nc.scalar.dma_start(xb[1][:], xc_d[:, :, 512:1024])
        nc.scalar.dma_start(xb[3][:], xc_d[:, :, 1536:2048])
        for i, ch in enumerate(chunks):
            if ch["mask_w"]:
                mo = ch["mask_off"]
                nc.sync.dma_start(mbs[i][:, :ch["mask_w"]],
                                  mk_d[:, mo:mo + ch["mask_w"]])
        nc.sync.dma_start(wo_sb[:], wo_d[:, :, :])
        nc.sync.dma_start(xb[2][:], xc_d[:, :, 1024:1536])
        for pi in range(NQ):
            nc.vector.memset(vb1[pi][:, :, HD:HD + 1], 1.0)

        def rope(cc, tg, raw, fs):
            rot = ps.tile([128, 512], F32, tag="rot", bufs=1, name="rot")
            nc.tensor.matmul(rot[:], rT[:], raw[:], start=True, stop=True)
            u = sp.tile([128, 512], BF16, tag="u", name="u")
            nc.vector.tensor_mul(u[:], rot[:], sinT[:, fs])
            m = sp.tile([128, 512], BF16, tag="m", name="m")
            nc.vector.tensor_mul(m[:], raw[:], cosT[:, fs])
            tgt = qbT[cc] if tg == "q" else kbT[cc]
            nc.vector.tensor_add(tgt[:, fs], m[:], u[:])

        def v_half(pc, half):
            psv = ps.tile([128, 512], F32, tag="v", bufs=1, name="psv")
            for ti in range(2):
                st = (2 * half + ti) * 128
                for k in range(KC):
                    nc.tensor.matmul(
                        psv[:, ti * 256:(ti + 1) * 256],
                        xb[pc][:, k, st:st + 128], wv_sb[:, k, :],
                        start=(k == 0), stop=(k == KC - 1))
            for ti in range(2):
                pi = 4 * pc + 2 * half + ti
                nc.vector.tensor_copy(vb1[pi][:, :, 0:HD],
                                      psv[:, ti * 256:(ti + 1) * 256])

        def qkv_band(pc):
            fs = slice(pc * 512, (pc + 1) * 512)
            quads = [(cc, w_sb, tg) for cc in range(CC)
                     for w_sb, tg in ((wq_sb, "q"), (wk_sb, "k"))]
            raws = {}
            for j, (cc, w_sb, tg) in enumerate(quads):
                psqk = ps.tile([128, 512], F32, tag="qk", bufs=2,
                               name="psqk")
                for k in range(KC):
                    nc.tensor.matmul(
                        psqk[:], w_sb[:, k, cc * 128:(cc + 1) * 128],
                        xb[pc][:, k, :], start=(k == 0),
                        stop=(k == KC - 1))
                raw = sp.tile([128, 512], BF16, tag="raw", name="raw")
                nc.scalar.copy(raw[:], psqk[:])
                raws[j] = raw
                # delay each quad's rotation by one quad so the PE never
                # waits on the ACT psum->sbuf copy
                if j >= 1:
                    pcc, _, ptg = quads[j - 1]
                    rope(pcc, ptg, raws[j - 1], fs)
            v_half(pc, 0)
            rope(quads[3][0], quads[3][2], raws[3], fs)
            v_half(pc, 1)
            if pc == 0:
                for j, (kt, rowruns, _users) in enumerate(sc["variants"]):
                    nc.gpsimd.memset(vgs[j][:], 0.0)
                    for lo, hi in rowruns:
                        nc.gpsimd.tensor_copy(vgs[j][lo:hi, :, :],
                                              vb1[kt][lo:hi, :, :])

        def attn_scores(ch, h, mb):
            """Scores + exp + mask for one head; returns pb per group."""
            cc, ho = h // 2, (h % 2) * 64
            pbg = []
            for gi, grp in enumerate(ch["groups"]):
                g0 = grp[0][4]
                gw = sum(c[3] for c in grp)
                scp = ps.tile([128, 512], F32, tag="sc", bufs=2,
                              name="scp")
                for kt, qlo, qhi, W, bo in grp:
                    go = bo - g0
                    nc.tensor.matmul(
                        scp[:, go:go + W],
                        kbT[cc][ho:ho + 64, kt * 128:(kt + 1) * 128],
                        qbT[cc][ho:ho + 64, qlo * 128:(qhi + 1) * 128],
                        start=True, stop=True)
                pb = bp.tile([128, 512], BF16, tag="pb", name="pb")
                nc.scalar.activation(pb[:, :gw], scp[:, :gw],
                                     mybir.ActivationFunctionType.Exp,
                                     bias=0.0, scale=SCALE)
                for go, w, moff in ch["mask_runs"][gi]:
                    nc.vector.tensor_mul(pb[:, go:go + w],
                                         pb[:, go:go + w],
                                         mb[:, moff:moff + w])
                pbg.append(pb)
            return pbg

        def attn_av(ch, h, pbg, ot_sb):
            cc, ho = h // 2, (h % 2) * 64
            nq = ch["qhi"] - ch["qlo"]
            av = ps.tile([128, 512], F32, tag="apw", bufs=2, name="av")
            # per-q-tile contiguous accumulation chains: interleaving
            # chains within one psum bank corrupts results on TRN2
            for qi in range(ch["qlo"], ch["qhi"]):
                co = (qi - ch["qlo"]) * 128
                for kt in kts_eff[qi]:
                    gi, go, qlo = ch["ktmap"][kt]
                    po = go + (qi - qlo) * 128
                    vi = sc["var_of"].get((qi, kt))
                    vb = vb1[kt] if vi is None else vgs[vi]
                    nc.tensor.matmul(
                        av[0:65, co:co + 128], vb[:, h:h + 1, :],
                        pbg[gi][:, po:po + 128],
                        start=(kt == kts_eff[qi][0]),
                        stop=(kt == kts_eff[qi][-1]))
            w = nq * 128
            lsb = lr.tile([1, 512], F32, tag="lsb", name="lsb")
            nc.vector.tensor_copy(lsb[:, :w], av[64:65, :w])
            rh = lr.tile([1, 512], F32, tag="rh", name="rh")
            nc.vector.reciprocal_approx_fast(rh[:, :w], lsb[:, :w])
            rb = lr.tile([64, 512], F32, tag="rb", name="rb")
            nc.gpsimd.partition_broadcast(rb[:, :w], rh[:, :w])
            nc.vector.tensor_mul(ot_sb[cc][ho:ho + 64, :w], av[0:64, :w],
                                 rb[:, :w])

        def attn_chunk(ci):
            ch = chunks[ci]
            mb = mbs[ci]
            nq = ch["qhi"] - ch["qlo"]
            ot_sb = {cc: otp.tile([128, 512], BF16, tag=f"ot{cc}",
                                  name=f"ot{cc}") for cc in range(CC)}
            ahead = min(ch["ahead"], HPC)
            pbgs = {h: attn_scores(ch, h, mb) for h in range(ahead)}
            for h in range(HPC):
                if h + ahead < HPC:
                    pbgs[h + ahead] = attn_scores(ch, h + ahead, mb)
                attn_av(ch, h, pbgs.pop(h), ot_sb)
            for qi4 in range(nq):
                ob = obp.tile([128, D], BF16, tag="ob", name="ob")
                for n2 in range(2):
                    pw = ps.tile([128, 512], F32, tag="apw", bufs=2,
                                 name="pw")
                    for cc2 in range(CC):
                        nc.tensor.matmul(
                            pw[:],
                            ot_sb[cc2][:, qi4 * 128:(qi4 + 1) * 128],
                            wo_sb[:, cc2, n2 * 512:(n2 + 1) * 512],
                            start=(cc2 == 0), stop=(cc2 == CC - 1))
                    if n2 == 0:
                        nc.scalar.copy(ob[:, 0:512], pw[:])
                    else:
                        nc.vector.tensor_copy(ob[:, 512:1024], pw[:])
                qi = ch["qlo"] + qi4
                nc.gpsimd.dma_start(out_d[qi * 128:(qi + 1) * 128, :],
                                    ob[:])

        # software-pipelined loop: attention lags QKV by one band
        qkv_band(0)
        qkv_band(1)
        attn_chunk(0)
        qkv_band(2)
        attn_chunk(1)
        qkv_band(3)
        for ci in range(2, len(chunks)):
            attn_chunk(ci)

        ctx.close()

    nc.compile()
    return nc


def _host_inputs(x, freqs_cos, freqs_sin, position_ids, mask01, sc,
                 Wq, Wk, Wv, Wo):
    """Per-core input maps (chunk-packed layouts, see _build_nc)."""
    import ml_dtypes
    bf = ml_dtypes.bfloat16

    def chunkpack(w):  # [nch*128, N] -> [128, nch, N]
        nch = w.shape[0] // 128
        return np.ascontiguousarray(
            w.reshape(nch, 128, w.shape[1]).transpose(1, 0, 2)).astype(bf)

    r64 = np.zeros((HD, HD), np.float32)
    for i in range(HD // 2):
        r64[2 * i, 2 * i + 1] = -1.0
        r64[2 * i + 1, 2 * i] = 1.0
    r128 = np.zeros((128, 128), np.float32)
    r128[:64, :64] = r64
    r128[64:, 64:] = r64
    rT = np.ascontiguousarray(r128.T).astype(bf)

    # packed transposed 0/1 mask for the DVE multiply
    mcols = max(sc["mask_cols"], 128)
    maskTc = np.zeros((128, mcols), bf)
    o = 0
    for qi, kt in sc["mask_blocks"]:
        maskTc[:, o:o + QT] = mask01[qi * QT:(qi + 1) * QT,
                                     kt * QT:(kt + 1) * QT].T
        o += QT
    assert o == sc["mask_cols"]

    in_maps = []
    for c in range(NCORES):
        b, g = c // HG, c % HG
        pos = np.clip(position_ids[b].astype(np.int64), 0,
                      freqs_cos.shape[0] - 1)
        cos_g = np.asarray(freqs_cos)[pos]  # [S, 32]
        sin_g = np.asarray(freqs_sin)[pos]
        cosT64 = np.repeat(cos_g.T, 2, axis=0)  # [64, S]
        sinT64 = np.repeat(sin_g.T, 2, axis=0)
        cs = slice(g * DC, (g + 1) * DC)
        in_maps.append({
            "xc": chunkpack(np.ascontiguousarray(x[b].T)),
            "wq": chunkpack(Wq[:, cs]),
            "wk": chunkpack(Wk[:, cs]),
            "wv": chunkpack(Wv[:, cs]),
            "wo": chunkpack(Wo[cs, :]),
            "cosT": np.concatenate([cosT64, cosT64], axis=0).astype(bf),
            "sinT": np.concatenate([sinT64, sinT64], axis=0).astype(bf),
            "rT": rT,
            "maskT": maskTc,
        })
    return in_maps


_CACHE = {}


def _get_nc(mask_key, sc):
    if mask_key not in _CACHE:
        _CACHE[mask_key] = _build_nc(sc)
    return _CACHE[mask_key]


def kernel(x, freqs_cos, freqs_sin, position_ids, bigbird_mask, Wq, Wk, Wv, Wo,
           _want_results=False, _trace=False, **trace_kwargs):
    x = np.asarray(x)
    mask = np.asarray(bigbird_mask).astype(bool)
    sc = _sched(mask)
    nc = _get_nc(mask.tobytes(), sc)
    in_maps = _host_inputs(
        x, np.asarray(freqs_cos), np.asarray(freqs_sin),
        np.asarray(position_ids), mask.astype(np.float32), sc,
        np.asarray(Wq), np.asarray(Wk), np.asarray(Wv), np.asarray(Wo),
    )
    res = bass_utils.run_bass_kernel_spmd(
        nc, in_maps, list(range(NCORES)), trace=_trace, **trace_kwargs
    )
    out = np.zeros((B, S, D), np.float32)
    for c in range(NCORES):
        out[c // HG] += res.results[c]["out"].astype(np.float32)
    if _want_results:
        return out, res
    return out
